# revision 25
# baseline (speedup 1.0000x reference)
"""LoRA linear kernel for Trainium2 (8 NeuronCores, SPMD data-parallel).

Computes out = x @ (A @ B) for
    x: [4, 2048, 4096] f32, A: [4096, 16] f32, B: [16, 4096] f32
by reassociating to (x @ A) @ B  (4.3 GFLOP instead of 274 GFLOP).

Sharding: x is split row-wise (batch*seq = 8192 rows -> 1024 rows/core).
A and B are replicated to every core. No collectives.

All matmul operands are fp16 (1 cycle/row on the PE array vs 4 for
fp32) and the output is shipped back as fp16 and upcast on the host,
halving HBM traffic in both directions. PSUM accumulation stays fp32.
Host-side prep blocks x (and A) so every DMA is per-partition
contiguous with >=512B lines.

The R=16 contraction/output dims would leave most of the PE array
idle, so row-blocks are processed concurrently in disjoint 32-wide
strips of the array via tile_position:
  stage 1 (col strips): strip g computes tT_g[16,128] = (x_blk_g @ A).T
      accumulating into PSUM partitions 32g..32g+16.
  stage 2 (row strips): strip g computes out_blk_g[128, dc] =
      tT_g.T @ B from SBUF partitions 32g..32g+16 (B replicated there).

The critical path is the PSUM->SBUF evacuation (~21us of copy work on
DVE+ACT) plus the in/out HBM streams, so the schedule keeps the copy
engines and the HBM port dense end to end:
  - A and B DMAs go first (tiny), then the whole 8 MiB x shard is
    prefetched up front in ~1 MiB pieces; input is never gated on
    compute.
  - the shard is processed as two 512-row sections with 4 concurrent
    PE strips each; wider strips keep enough copy work per stage-2
    unit to hide semaphore/wake latencies on the copy engines.
  - stage 2 runs dc-outer so strip matmuls issue concurrently, with 7
    single-bank PSUM tiles (+1 stage-1 bank = all 8) so matmul issue
    runs ahead of the copies.
  - generously-spaced monotone sim-time floors pin every engine's
    static stream to the emission order, interleaving each section's
    stage 1 into the previous section's stage-2 PSUM-wait bubbles.
  - output DMAs fire per half-block as copies land (quarters for the
    final section), keeping the output stream flowing instead of
    bursting at section boundaries. They stay on the Sync HWDGE queue:
    GPSIMD/SWDGE triggers were measured to cost a ~5us ring-drain in
    the NEFF epilogue.
"""

import numpy as np

import concourse.bass as bass
import concourse.bacc as bacc
import concourse.mybir as mybir
from concourse.tile import TileContext
from concourse.bass_utils import run_bass_kernel_spmd

N_CORES = 8
BATCH, SEQ, D_IN, D_OUT, R = 4, 2048, 4096, 4096, 16
ROWS = BATCH * SEQ              # 8192
RPC = ROWS // N_CORES           # 1024 rows per core
KC = D_IN // 128                # 32 contraction chunks of 128
DC = 512                        # d_out columns per stage-2 matmul (PSUM bank)
NDC = D_OUT // DC               # 8

F32 = mybir.dt.float32
F16 = mybir.dt.float16

# (row0, nblk, kc-splits): sections processed in order; each section
# covers rows [row0, row0 + 128*nblk) with nblk concurrent PE strips,
# and its input arrives in len(splits) pieces covering those kc ranges.
SECTIONS = [
    (0, 4, [(0, 8), (8, 16), (16, 24), (24, 32)]),
    (512, 4, [(0, 8), (8, 16), (16, 24), (24, 32)]),
]
XCOLS = RPC * KC * 128 // 128   # per-partition fp16 elements of x

_cache = {}


def _host_pack_x(xs):
    """Pack one core's [RPC, D_IN] shard into the flat per-partition
    layout consumed by the kernel's section pieces."""
    blocks = []
    for row0, nblk, splits in SECTIONS:
        rchunk = 128 * nblk
        rows = xs[row0:row0 + rchunk]
        for c0, c1 in splits:
            blk = rows[:, c0 * 128:(c1) * 128]
            blk = blk.reshape(rchunk, c1 - c0, 128).transpose(2, 1, 0)
            blocks.append(blk.reshape(128, -1))
    return np.ascontiguousarray(np.concatenate(blocks, axis=1)
                                ).astype(np.float16)


def _build(mm_dtype=F16):
    nc = bacc.Bacc("TRN2", target_bir_lowering=False)
    xTf = nc.dram_tensor("xTf", [128, XCOLS], mm_dtype,
                         kind="ExternalInput")
    # Ab[p, c, r] = A[c*128 + p, r]  (host-blocked: contiguous 1 KiB/line)
    Ab = nc.dram_tensor("Ab", [128, KC, R], mm_dtype, kind="ExternalInput")
    Bw = nc.dram_tensor("Bw", [R, D_OUT], mm_dtype, kind="ExternalInput")
    out = nc.dram_tensor("out", [RPC, D_OUT], mm_dtype,
                         kind="ExternalOutput")

    with TileContext(nc) as tc:
        with (
            tc.tile_pool(name="consts", bufs=1) as cpool,
            tc.tile_pool(name="xin", bufs=8) as xpool,
            tc.tile_pool(name="tbuf", bufs=2) as tpool,
            tc.tile_pool(name="obuf", bufs=8) as opool,
            tc.tile_pool(name="pt", bufs=1, space="PSUM") as ptpool,
            tc.tile_pool(name="po", bufs=7, space="PSUM") as popool,
        ):
            seq = [0]

            def floor():
                tc.tile_set_cur_wait(0.01 * seq[0])
                seq[0] += 1

            a_tile = cpool.tile([128, KC, R], mm_dtype)
            nc.sync.dma_start(out=a_tile[:], in_=Ab[:, :, :])
            # the entire input shard, issued up front; B is queued
            # right after section 0's pieces (it is not needed until
            # stage 2, ~5us after section 0's last piece lands)
            b4 = cpool.tile([128, D_OUT], mm_dtype)
            xts = {}
            off = 0
            for si, (row0, nblk, splits) in enumerate(SECTIONS):
                rchunk = 128 * nblk
                for pi, (c0, c1) in enumerate(splits):
                    cs = c1 - c0
                    xt = xpool.tile([128, cs, rchunk], mm_dtype,
                                    name="xt", tag="xt")
                    src = xTf[:, off:off + cs * rchunk]
                    nc.sync.dma_start(
                        out=xt[:],
                        in_=src.rearrange("p (c n) -> p c n", c=cs))
                    xts[si, pi] = xt
                    off += cs * rchunk
                if si == 0:
                    for g in range(4):
                        nc.sync.dma_start(out=b4[32 * g:32 * g + R, :],
                                          in_=Bw[:, :])

            pts = {}

            def s1_piece(si, pi):
                row0, nblk, splits = SECTIONS[si]
                c0, c1 = splits[pi]
                if pi == 0:
                    pts[si] = ptpool.tile([128, 128], F32, name="pt",
                                          tag="pt")
                pt = pts[si]
                xt = xts[si, pi]
                for c in range(c1 - c0):
                    for g in range(nblk):
                        nc.tensor.matmul(
                            pt[32 * g:32 * g + R, :],
                            a_tile[:, c0 + c, :],
                            xt[:, c, 128 * g:128 * (g + 1)],
                            start=(c0 + c == 0),
                            stop=(c0 + c == KC - 1),
                            tile_position=(0, 32 * g),
                            skip_group_check=True,
                        )

            def s1_cast(si):
                tT4 = tpool.tile([128, 128], mm_dtype)
                nc.vector.tensor_copy(tT4[:], pts[si][:])
                return tT4

            # section 0 stage 1: pieces as their DMAs land
            for pi in range(len(SECTIONS[0][2])):
                floor()
                s1_piece(0, pi)
            floor()
            tT = s1_cast(0)

            nsec = len(SECTIONS)
            for si, (row0, nblk, splits) in enumerate(SECTIONS):
                osbs = [opool.tile([128, D_OUT], mm_dtype, name=f"osb{g}",
                                   tag="osb") for g in range(nblk)]
                nxt = list(range(len(SECTIONS[si + 1][2]))) \
                    if si + 1 < nsec else []
                last = si == nsec - 1
                for j in range(NDC // 2):
                    # stage-2 unit j: dc pair (2j, 2j+1), all strips
                    floor()
                    for dc in (2 * j, 2 * j + 1):
                        for g in range(nblk):
                            po = popool.tile([128, DC], F32, name="po",
                                             tag="po")
                            nc.tensor.matmul(
                                po[:],
                                tT[32 * g:32 * g + R, :],
                                b4[32 * g:32 * g + R,
                                   dc * DC:(dc + 1) * DC],
                                start=True,
                                stop=True,
                                tile_position=(32 * g, 0),
                                skip_group_check=True,
                            )
                            dst = osbs[g][:, dc * DC:(dc + 1) * DC]
                            if (dc + g) % 2 == 0:
                                nc.vector.tensor_copy(dst, po[:])
                            else:
                                nc.scalar.copy(out=dst, in_=po[:])
                    # output DMAs (SWDGE/GPSIMD: keeps triggers off the
                    # Sync queue) as soon as the columns are complete
                    spans = []
                    if j == 1:
                        spans = [(0, 4 * DC)]
                    elif last and j == 2:
                        spans = [(4 * DC, 6 * DC)]
                    elif j == 3:
                        spans = [(6 * DC, 8 * DC)] if last \
                            else [(4 * DC, 8 * DC)]
                    for c0_, c1_ in spans:
                        for g in range(nblk):
                            r0 = row0 + 128 * g
                            nc.sync.dma_start(
                                out=out[r0:r0 + 128, c0_:c1_],
                                in_=osbs[g][:, c0_:c1_])
                    # interleave the NEXT section's stage-1 pieces into
                    # this unit's PSUM-wait bubbles (piece p goes after
                    # unit p*4//npieces)
                    while nxt and nxt[0] * 4 // len(SECTIONS[si + 1][2]) == j:
                        floor()
                        s1_piece(si + 1, nxt.pop(0))
                if si + 1 < nsec:
                    floor()
                    tT = s1_cast(si + 1)
    nc.compile()
    return nc


def _get_nc(mm_dtype=F16):
    key = (str(mm_dtype),)
    if key not in _cache:
        _cache[key] = _build(mm_dtype)
    return _cache[key]


def kernel(x, A, B, trace=False, mm_dtype=None):
    if mm_dtype is None:
        mm_dtype = F16
    x = np.asarray(x, dtype=np.float32)
    Ah = np.ascontiguousarray(
        np.asarray(A).reshape(KC, 128, R).transpose(1, 0, 2)
    ).astype(np.float16)
    Bh = np.ascontiguousarray(np.asarray(B)).astype(np.float16)
    xf = x.reshape(ROWS, D_IN)

    nc = _get_nc(mm_dtype)
    in_maps = []
    for i in range(N_CORES):
        xs = xf[i * RPC:(i + 1) * RPC]                 # [1024, 4096]
        in_maps.append({"xTf": _host_pack_x(xs), "Ab": Ah, "Bw": Bh})

    res = run_bass_kernel_spmd(nc, in_maps, list(range(N_CORES)), trace=trace)
    outs = [res.results[i]["out"] for i in range(N_CORES)]
    full = np.concatenate(outs, axis=0).astype(np.float32)
    full = full.reshape(BATCH, SEQ, D_OUT)
    if trace:
        kernel.last_exec_time_ns = res.exec_time_ns
        kernel.last_results = res
    return full
